# revision 14
# baseline (speedup 1.0000x reference)
"""Causal multi-head attention (B=4, T=2048, D=1024, H=16) on 8 trn2 cores.

Sharding: core c -> (batch b = c//2, head-group g = c%2) -> 8 heads/core.

Per-core schedule:
  - One FLAT software pipeline over (qg, pair-pair-block, k-chunk): two
    interleaved pair-streams, each with per-head [128,512] score tiles on a
    4-buf PSUM rotation -> 4 exp->QK round-trips overlap and blocks chain
    into each other without pipeline drains.
  - All projection matmuls (QKV, output) run in FP8 E4M3 with DoubleRow
    (2 k-tiles per pass) as "filler" pumped between attention chunks.
    Weights are scaled x8 on host to stay in e4m3 normal range; the softmax
    1/sqrt(dh) and the x64 output compensation fold into the ACT exp scale
    and the host-side gather.
  - V carries 64 ones-columns so the AV matmul materializes the softmax
    denominator replicated across PSUM partitions 64..127; normalization is
    tensor_copy + reciprocal_approx_fast + two fused multiply-evictions,
    dribbled into the following iterations.
  - causal mask applied by gpsimd affine_select directly on ex.
"""

import numpy as np
import ml_dtypes

import concourse.bass as bass  # noqa: F401  (bass types via bacc)
import concourse.bacc as bacc
import concourse.mybir as mybir
import concourse.tile as tile
from concourse.bass_utils import run_bass_kernel_spmd

B, T, D = 4, 2048, 1024
H, DH = 16, 64
N_CORES = 8
HPC = 8      # heads per core
PAIRS = HPC // 2
BF = mybir.dt.bfloat16
F32 = mybir.dt.float32
F8 = mybir.dt.float8e4
BF_NP = ml_dtypes.bfloat16
F8_NP = ml_dtypes.float8_e4m3

FP8 = False
WS = 8.0 if FP8 else 1.0           # host-side weight scale (e4m3 range)
EXP_SCALE = 1.0 / (np.sqrt(DH) * WS * WS)
Y_DIV = WS * WS                    # host-side output divisor
XW = F8 if FP8 else BF
DR = mybir.MatmulPerfMode.DoubleRow if FP8 else None

TQ = 512     # q block (free dim)
TK = 128     # k block (partition dim)
NQG = T // TQ
NKC = T // TK


def build_nc():
    nc = bacc.Bacc(
        "TRN2",
        target_bir_lowering=False,
        debug=False,
        enable_asserts=True,
        num_devices=N_CORES,
    )
    xT = nc.dram_tensor("xT", [D, T], XW, kind="ExternalInput")
    wq = nc.dram_tensor("wq", [D, 512], XW, kind="ExternalInput")
    wk = nc.dram_tensor("wk", [D, 512], XW, kind="ExternalInput")
    wv = nc.dram_tensor("wv", [D, 512], XW, kind="ExternalInput")
    wp = nc.dram_tensor("wp", [512, D], XW, kind="ExternalInput")
    y = nc.dram_tensor("y", [T, D], F32, kind="ExternalOutput")

    with tile.TileContext(nc) as tc:
        with (
            tc.tile_pool(name="pers", bufs=1) as pers,
            tc.tile_pool(name="work", bufs=1) as work,
            tc.tile_pool(name="ps", bufs=1, space="PSUM") as pp,
        ):
            # ---- persistent SBUF ----
            xT_sb = pers.tile([128, 8, T], XW, tag="xT", name="xT_sb")
            wq_sb = pers.tile([128, 8, 512], XW, tag="wq", name="wq_sb")
            wk_sb = pers.tile([128, 8, 512], XW, tag="wk", name="wk_sb")
            wv_sb = pers.tile([128, 8, 512], XW, tag="wv", name="wv_sb")
            wp_sb = pers.tile([128, 4, D], XW, tag="wp", name="wp_sb")
            # V in token-major layout; cols 64..127 are ones so the AV
            # matmul writes the denominator to psO partitions 64..127.
            vext = pers.tile([128, NKC, HPC, 128], BF, tag="vext", name="vext")
            qt_all = pers.tile([128, PAIRS, T], BF, tag="qt", name="qt_all")
            kt_all = pers.tile([128, PAIRS, T], BF, tag="kt", name="kt_all")
            # normalized attention outputs, d-major: [pair-chan, pair, tok]
            outT = pers.tile([128, PAIRS, T], XW, tag="outT", name="outT")

            # ---- loads (ordered so qg0 pair0/1 QK^T deps land first) ----
            for dc in range(8):
                nc.sync.dma_start(wq_sb[:, dc, :], wq[dc * 128:(dc + 1) * 128, :])
                nc.sync.dma_start(wk_sb[:, dc, :], wk[dc * 128:(dc + 1) * 128, :])
                nc.sync.dma_start(xT_sb[:, dc, 0:512], xT[dc * 128:(dc + 1) * 128, 0:512])
            for dc in range(8):
                nc.sync.dma_start(wv_sb[:, dc, :], wv[dc * 128:(dc + 1) * 128, :])
            for dc in range(8):
                nc.sync.dma_start(xT_sb[:, dc, 512:T], xT[dc * 128:(dc + 1) * 128, 512:T])
            for cc in range(4):
                nc.sync.dma_start(wp_sb[:, cc, :], wp[cc * 128:(cc + 1) * 128, :])
            nc.gpsimd.memset(vext[:, :, :, 64:128], 1.0)

            # ---- filler machinery (all PSUM tiles are 1-bank [128,512]) ----
            filler = []
            fptr = [0]

            def pump(n=1):
                while n > 0 and fptr[0] < len(filler):
                    filler[fptr[0]]()
                    fptr[0] += 1
                    n -= 1

            def drain_until(idx):
                while fptr[0] <= idx:
                    if fptr[0] >= len(filler):
                        return
                    filler[fptr[0]]()
                    fptr[0] += 1

            def dr_chain(ps, lhs_t, lhs_cols, rhs_t, rhs_cols, d2lo, ntiles):
                """Two DoubleRow (or four bf16) accumulation matmuls."""
                if FP8:
                    for d2 in range(d2lo, d2lo + 2):
                        nc.tensor.matmul(
                            ps[:, :],
                            lhs_t[:, 2 * d2:2 * d2 + 2, lhs_cols],
                            rhs_t[:, 2 * d2:2 * d2 + 2, rhs_cols],
                            start=(d2 == 0), stop=(d2 == ntiles // 2 - 1),
                            perf_mode=DR,
                        )
                else:
                    for dc in range(2 * d2lo, 2 * d2lo + 4):
                        nc.tensor.matmul(
                            ps[:, :],
                            lhs_t[:, dc, lhs_cols],
                            rhs_t[:, dc, rhs_cols],
                            start=(dc == 0), stop=(dc == ntiles - 1),
                        )

            # V projection of one token chunk tk (all heads)
            def v_tile_closures(tk):
                st = {}

                def mm(d2lo):
                    if d2lo == 0:
                        st["ps"] = pp.tile([128, 512], F32, tag="sc", bufs=4,
                                           name="ps_v")
                    dr_chain(st["ps"], xT_sb, slice(tk * 128, (tk + 1) * 128),
                             wv_sb, slice(0, 512), d2lo, 8)

                def cast():
                    nc.vector.tensor_copy(
                        vext[:, tk, :, 0:64],
                        st["ps"].rearrange("p (h d) -> p h d", d=64),
                    )

                return [lambda: mm(0), lambda: mm(2), cast]

            # Q^T or K^T projection for (qg, pair), d-major, 512 tokens
            def qkt_closures(qg, pair):
                sl = slice(qg * TQ, (qg + 1) * TQ)

                def half(w_sb, dst):
                    st = {}

                    def mm(d2lo):
                        if d2lo == 0:
                            st["ps"] = pp.tile([128, 512], F32, tag="sc",
                                               bufs=4, name="ps_qk")
                        dr_chain(st["ps"], w_sb,
                                 slice(pair * 128, (pair + 1) * 128),
                                 xT_sb, sl, d2lo, 8)

                    def cast():
                        nc.vector.tensor_copy(dst[:, pair, sl], st["ps"][:, :])

                    return [lambda: mm(0), lambda: mm(2), cast]

                return half(wq_sb, qt_all) + half(wk_sb, kt_all)

            # output projection for one (token chunk, D-half)
            def proj_closures(tk, nb):
                st = {}

                def mm():
                    st["ps"] = pp.tile([128, 512], F32, tag="sc", bufs=4,
                                       name="ps_y")
                    if FP8:
                        for c2 in range(2):
                            nc.tensor.matmul(
                                st["ps"][:, :],
                                outT[:, 2 * c2:2 * c2 + 2,
                                     tk * 128:(tk + 1) * 128],
                                wp_sb[:, 2 * c2:2 * c2 + 2,
                                      nb * 512:(nb + 1) * 512],
                                start=(c2 == 0), stop=(c2 == 1),
                                perf_mode=DR,
                            )
                    else:
                        for cc in range(4):
                            nc.tensor.matmul(
                                st["ps"][:, :],
                                outT[:, cc, tk * 128:(tk + 1) * 128],
                                wp_sb[:, cc, nb * 512:(nb + 1) * 512],
                                start=(cc == 0), stop=(cc == 3),
                            )

                def evict():
                    y_sb = work.tile([128, 512], F32, tag="ysb", bufs=3,
                                     name="y_sb")
                    nc.vector.tensor_copy(y_sb[:, :], st["ps"][:, :])
                    nc.sync.dma_start(
                        y[tk * 128:(tk + 1) * 128, nb * 512:(nb + 1) * 512],
                        y_sb[:, :])

                return [mm, evict]

            # prefix: only qg0 pair0+pair1 QK^T emitted directly
            for p in (0, 1):
                for cl in qkt_closures(0, p):
                    cl()

            # V closures live on their own demand-driven queue: AV(kc) drains
            # V up to chunk kc, so V never causes a block-start burst
            vq = []
            vq_mark = {}
            vptr = [0]
            for tk in range(NKC):
                vq.extend(v_tile_closures(tk))
                vq_mark[tk] = len(vq) - 1

            def drain_v(tk):
                tk = min(tk, NKC - 1)
                while vptr[0] <= vq_mark[tk]:
                    vq[vptr[0]]()
                    vptr[0] += 1

            # main filler queue (QK^T projections + output projections)
            marker = {(0, 0): -1, (0, 1): -1}
            for p in (2, 3):
                filler.extend(qkt_closures(0, p))
                marker[(0, p)] = len(filler) - 1
            for qg in (1, 2, 3):
                for p in range(4):
                    filler.extend(qkt_closures(qg, p))
                    marker[(qg, p)] = len(filler) - 1

            # ---- attention: one flat pipeline over (block, k-chunk) ----
            blocks = [(qg, blk) for qg in range(NQG) for blk in (0, 1)]

            def bparams(bi):
                qg, blk = blocks[bi]
                return qg, blk, qg * (TQ // TK), (qg + 1) * (TQ // TK)

            normq = []
            proj_pending = [None]
            psO_of = {}

            def alloc_psO(bi):
                psO_of[bi] = (
                    pp.tile([128, 1024], F32, tag="o", bufs=2, name="psO_A"),
                    pp.tile([128, 1024], F32, tag="o", bufs=2, name="psO_B"),
                )

            def subs_of(bi):
                qg, blk, _, _ = bparams(bi)
                pA, pB = 2 * blk, 2 * blk + 1
                a, b = psO_of[bi]
                return [(pA, 0, a), (pA, 1, a), (pB, 0, b), (pB, 1, b)]

            def qk1(bi, pair, h, kc):
                qg, _, noff, _ = bparams(bi)
                off = max(0, kc - noff) * TK
                ps_s = pp.tile([128, 512], F32, tag="sc", bufs=4, name="ps_s")
                nc.tensor.matmul(
                    ps_s[:, off:512],
                    kt_all[h * 64:(h + 1) * 64, pair, kc * TK:(kc + 1) * TK],
                    qt_all[h * 64:(h + 1) * 64, pair,
                           qg * TQ + off:(qg + 1) * TQ],
                    start=True, stop=True,
                )
                return ps_s

            def exp_mask1(bi, pair, h, kc, ps_s):
                qg, _, noff, _ = bparams(bi)
                off = max(0, kc - noff) * TK
                ex = work.tile([128, 512], BF, tag="ex", bufs=16, name="ex")
                nc.scalar.activation(
                    ex[:, off:512], ps_s[:, off:512],
                    mybir.ActivationFunctionType.Exp,
                    scale=float(EXP_SCALE),
                )
                if kc >= noff:
                    nc.gpsimd.affine_select(
                        ex[:, off:off + TK], ex[:, off:off + TK],
                        pattern=[[1, TK]],
                        compare_op=mybir.AluOpType.is_ge,
                        fill=0.0, base=0, channel_multiplier=-1,
                    )
                return ex

            def av1(bi, pair, h, kc, ex, psO):
                qg, _, noff, kmax = bparams(bi)
                off = max(0, kc - noff) * TK
                nc.tensor.matmul(
                    psO[:, h * 512 + off:(h + 1) * 512],
                    vext[:, kc, pair * 2 + h, :],
                    ex[:, off:512],
                    start=(kc == 0), stop=(kc == kmax - 1),
                    skip_group_check=True,
                )

            def finish_pair(bi, pair, psO, tail=False):
                qg, _, _, _ = bparams(bi)
                st = {}

                def c_den():
                    st["den"] = work.tile([64, 1024], F32, tag="den",
                                          bufs=2, name="den_sb")
                    if tail:
                        nc.scalar.copy(st["den"][:, :], psO[64:128, :])
                    else:
                        nc.vector.tensor_copy(st["den"][:, :], psO[64:128, :])

                def c_recip():
                    st["bcr"] = work.tile([64, 1024], F32, tag="bcr",
                                          bufs=2, name="bcr")
                    nc.vector.reciprocal_approx_fast(st["bcr"][:, :],
                                                     st["den"][:, :])

                def c_mult(h):
                    nc.vector.tensor_mul(
                        outT[h * 64:(h + 1) * 64, pair,
                             qg * TQ:(qg + 1) * TQ],
                        psO[0:64, h * 512:(h + 1) * 512],
                        st["bcr"][0:64, h * 512:(h + 1) * 512],
                    )

                normq.extend([c_den, c_recip,
                              lambda: c_mult(0), lambda: c_mult(1)])

            NB = len(blocks)
            alloc_psO(0)
            cur = [qk1(0, p, h, 0) for (p, h, _) in subs_of(0)]
            cur_next = None
            prev = None  # (bi, kc, exs)
            git = 0      # global iteration counter

            for bi in range(NB):
                qg, blk, noff, kmax = bparams(bi)
                if blk == 0 and qg > 0:
                    proj_pending[0] = list(range((qg - 1) * 4, qg * 4))
                for kc in range(kmax):
                    # V projections the upcoming AVs need, drained on demand
                    drain_v(kc)
                    exs = [exp_mask1(bi, p, h, kc, cur[i])
                           for i, (p, h, _) in enumerate(subs_of(bi))]
                    if prev is not None:
                        pbi, pkc, pexs = prev
                        for i, (p, h, psO) in enumerate(subs_of(pbi)):
                            av1(pbi, p, h, pkc, pexs[i], psO)
                        if pkc == bparams(pbi)[3] - 1:
                            _, pblk, _, _ = bparams(pbi)
                            finish_pair(pbi, 2 * pblk, psO_of[pbi][0])
                            finish_pair(pbi, 2 * pblk + 1, psO_of[pbi][1])
                    for _ in range(4):
                        if normq:
                            normq.pop(0)()
                    if not normq and proj_pending[0] is not None:
                        for tk in proj_pending[0]:
                            for nb in (0, 1):
                                filler.extend(proj_closures(tk, nb))
                        proj_pending[0] = None
                    pump(4 if git < 4 else (3 if qg <= 1 else 2))
                    # two-iteration lookahead: next block's first QKs are
                    # emitted one iteration before this block ends
                    if kc == kmax - 2 and bi + 1 < NB:
                        nqg, nblk, _, _ = bparams(bi + 1)
                        drain_until(max(marker[(nqg, 2 * nblk)],
                                        marker[(nqg, 2 * nblk + 1)]))
                    if kc + 1 < kmax:
                        cur = [qk1(bi, p, h, kc + 1)
                               for (p, h, _) in subs_of(bi)]
                        if kc + 1 == kmax - 1 and bi + 1 < NB:
                            alloc_psO(bi + 1)
                            cur_next = [qk1(bi + 1, p, h, 0)
                                        for (p, h, _) in subs_of(bi + 1)]
                    elif bi + 1 < NB:
                        cur = cur_next
                    else:
                        cur = None
                    prev = (bi, kc, exs)
                    git += 1

            # ---- tail: last AVs + last normalize + last qg's projection ----
            pbi, pkc, pexs = prev
            for i, (p, h, psO) in enumerate(subs_of(pbi)):
                av1(pbi, p, h, pkc, pexs[i], psO)
            finish_pair(pbi, 2, psO_of[pbi][0], tail=True)
            finish_pair(pbi, 3, psO_of[pbi][1])
            while normq:
                normq.pop(0)()
            for tk in range(12, 16):
                for nb in (0, 1):
                    filler.extend(proj_closures(tk, nb))
            drain_until(len(filler) - 1)

    nc.compile()
    return nc


_NC_CACHE = None


def _get_nc():
    global _NC_CACHE
    if _NC_CACHE is None:
        _NC_CACHE = build_nc()
    return _NC_CACHE


def make_in_maps(x, w_qkv, w_proj):
    """Host-side sharding: core c -> (batch c//2, head-group c%2)."""
    np_dt = F8_NP if FP8 else BF_NP
    in_maps = []
    for c in range(N_CORES):
        b, g = divmod(c, 2)
        sl = slice(g * 512, (g + 1) * 512)
        xT = np.ascontiguousarray(x[b].T).astype(np_dt)
        wq = (w_qkv[:, 0 * D:1 * D][:, sl] * WS).astype(np_dt)
        wk = (w_qkv[:, 1 * D:2 * D][:, sl] * WS).astype(np_dt)
        wv = (w_qkv[:, 2 * D:3 * D][:, sl] * WS).astype(np_dt)
        wp = (np.ascontiguousarray(w_proj[sl, :]) * WS).astype(np_dt)
        in_maps.append({"xT": xT, "wq": wq, "wk": wk, "wv": wv, "wp": wp})
    return in_maps


def kernel(x, w_qkv, w_proj, _trace=False, _tmpdir=None):
    x = np.asarray(x, dtype=np.float32)
    w_qkv = np.asarray(w_qkv, dtype=np.float32)
    w_proj = np.asarray(w_proj, dtype=np.float32)
    nc = _get_nc()
    in_maps = make_in_maps(x, w_qkv, w_proj)
    res = run_bass_kernel_spmd(
        nc, in_maps, core_ids=list(range(N_CORES)), trace=_trace, tmpdir=_tmpdir
    )
    out = np.empty((B, T, D), dtype=np.float32)
    inv = np.float32(1.0 / Y_DIV)
    for b in range(B):
        out[b] = (res.results[2 * b]["y"] + res.results[2 * b + 1]["y"]) * inv
    if _trace:
        kernel._last_results = res
    return out


# revision 15
# speedup vs baseline: 1.0017x; 1.0017x over previous
"""Causal multi-head attention (B=4, T=2048, D=1024, H=16) on 8 trn2 cores.

Sharding: core c -> (batch b = c//2, head-group g = c%2) -> 8 heads/core.

Per-core schedule:
  - One FLAT software pipeline over (qg, pair-pair-block, k-chunk): two
    interleaved pair-streams, each with per-head [128,512] score tiles on a
    4-buf PSUM rotation -> 4 exp->QK round-trips overlap and blocks chain
    into each other without pipeline drains.
  - All projection matmuls (QKV, output) run in FP8 E4M3 with DoubleRow
    (2 k-tiles per pass) as "filler" pumped between attention chunks.
    Weights are scaled x8 on host to stay in e4m3 normal range; the softmax
    1/sqrt(dh) and the x64 output compensation fold into the ACT exp scale
    and the host-side gather.
  - V carries 64 ones-columns so the AV matmul materializes the softmax
    denominator replicated across PSUM partitions 64..127; normalization is
    tensor_copy + reciprocal_approx_fast + two fused multiply-evictions,
    dribbled into the following iterations.
  - causal mask applied by gpsimd affine_select directly on ex.
"""

import numpy as np
import ml_dtypes

import concourse.bass as bass  # noqa: F401  (bass types via bacc)
import concourse.bacc as bacc
import concourse.mybir as mybir
import concourse.tile as tile
from concourse.bass_utils import run_bass_kernel_spmd

B, T, D = 4, 2048, 1024
H, DH = 16, 64
N_CORES = 8
HPC = 8      # heads per core
PAIRS = HPC // 2
BF = mybir.dt.bfloat16
F32 = mybir.dt.float32
F8 = mybir.dt.float8e4
BF_NP = ml_dtypes.bfloat16
F8_NP = ml_dtypes.float8_e4m3

FP8 = False
WS = 8.0 if FP8 else 1.0           # host-side weight scale (e4m3 range)
EXP_SCALE = 1.0 / (np.sqrt(DH) * WS * WS)
Y_DIV = WS * WS                    # host-side output divisor
XW = F8 if FP8 else BF
DR = mybir.MatmulPerfMode.DoubleRow if FP8 else None

TQ = 512     # q block (free dim)
TK = 128     # k block (partition dim)
NQG = T // TQ
NKC = T // TK


def build_nc():
    nc = bacc.Bacc(
        "TRN2",
        target_bir_lowering=False,
        debug=False,
        enable_asserts=True,
        num_devices=N_CORES,
    )
    xT = nc.dram_tensor("xT", [D, T], XW, kind="ExternalInput")
    wq = nc.dram_tensor("wq", [D, 512], XW, kind="ExternalInput")
    wk = nc.dram_tensor("wk", [D, 512], XW, kind="ExternalInput")
    wv = nc.dram_tensor("wv", [D, 512], XW, kind="ExternalInput")
    wp = nc.dram_tensor("wp", [512, D], XW, kind="ExternalInput")
    y = nc.dram_tensor("y", [T, D], F32, kind="ExternalOutput")

    with tile.TileContext(nc) as tc:
        with (
            tc.tile_pool(name="pers", bufs=1) as pers,
            tc.tile_pool(name="work", bufs=1) as work,
            tc.tile_pool(name="ps", bufs=1, space="PSUM") as pp,
        ):
            # ---- persistent SBUF ----
            xT_sb = pers.tile([128, 8, T], XW, tag="xT", name="xT_sb")
            wq_sb = pers.tile([128, 8, 512], XW, tag="wq", name="wq_sb")
            wk_sb = pers.tile([128, 8, 512], XW, tag="wk", name="wk_sb")
            wv_sb = pers.tile([128, 8, 512], XW, tag="wv", name="wv_sb")
            wp_sb = pers.tile([128, 4, D], XW, tag="wp", name="wp_sb")
            # V in token-major layout; cols 64..127 are ones so the AV
            # matmul writes the denominator to psO partitions 64..127.
            vext = pers.tile([128, NKC, HPC, 128], BF, tag="vext", name="vext")
            qt_all = pers.tile([128, PAIRS, T], BF, tag="qt", name="qt_all")
            kt_all = pers.tile([128, PAIRS, T], BF, tag="kt", name="kt_all")
            # normalized attention outputs, d-major: [pair-chan, pair, tok]
            outT = pers.tile([128, PAIRS, T], XW, tag="outT", name="outT")

            # ---- loads (ordered so qg0 pair0/1 QK^T deps land first) ----
            for dc in range(8):
                nc.sync.dma_start(wq_sb[:, dc, :], wq[dc * 128:(dc + 1) * 128, :])
                nc.sync.dma_start(wk_sb[:, dc, :], wk[dc * 128:(dc + 1) * 128, :])
                nc.sync.dma_start(xT_sb[:, dc, 0:512], xT[dc * 128:(dc + 1) * 128, 0:512])
            for dc in range(8):
                nc.sync.dma_start(wv_sb[:, dc, :], wv[dc * 128:(dc + 1) * 128, :])
            for dc in range(8):
                nc.sync.dma_start(xT_sb[:, dc, 512:T], xT[dc * 128:(dc + 1) * 128, 512:T])
            for cc in range(4):
                nc.sync.dma_start(wp_sb[:, cc, :], wp[cc * 128:(cc + 1) * 128, :])
            nc.gpsimd.memset(vext[:, :, :, 64:128], 1.0)

            # ---- filler machinery (all PSUM tiles are 1-bank [128,512]) ----
            filler = []
            fptr = [0]

            def pump(n=1):
                while n > 0 and fptr[0] < len(filler):
                    filler[fptr[0]]()
                    fptr[0] += 1
                    n -= 1

            def drain_until(idx):
                while fptr[0] <= idx:
                    if fptr[0] >= len(filler):
                        return
                    filler[fptr[0]]()
                    fptr[0] += 1

            def dr_chain(ps, lhs_t, lhs_cols, rhs_t, rhs_cols, d2lo, ntiles):
                """Two DoubleRow (or four bf16) accumulation matmuls."""
                if FP8:
                    for d2 in range(d2lo, d2lo + 2):
                        nc.tensor.matmul(
                            ps[:, :],
                            lhs_t[:, 2 * d2:2 * d2 + 2, lhs_cols],
                            rhs_t[:, 2 * d2:2 * d2 + 2, rhs_cols],
                            start=(d2 == 0), stop=(d2 == ntiles // 2 - 1),
                            perf_mode=DR,
                        )
                else:
                    for dc in range(2 * d2lo, 2 * d2lo + 4):
                        nc.tensor.matmul(
                            ps[:, :],
                            lhs_t[:, dc, lhs_cols],
                            rhs_t[:, dc, rhs_cols],
                            start=(dc == 0), stop=(dc == ntiles - 1),
                        )

            # V projection of one token chunk tk (all heads)
            def v_tile_closures(tk):
                st = {}

                def mm(d2lo):
                    if d2lo == 0:
                        st["ps"] = pp.tile([128, 512], F32, tag="sc", bufs=4,
                                           name="ps_v")
                    dr_chain(st["ps"], xT_sb, slice(tk * 128, (tk + 1) * 128),
                             wv_sb, slice(0, 512), d2lo, 8)

                def cast():
                    nc.vector.tensor_copy(
                        vext[:, tk, :, 0:64],
                        st["ps"].rearrange("p (h d) -> p h d", d=64),
                    )

                return [lambda: mm(0), lambda: mm(2), cast]

            # Q^T or K^T projection for (qg, pair), d-major, 512 tokens
            def qkt_closures(qg, pair):
                sl = slice(qg * TQ, (qg + 1) * TQ)

                def half(w_sb, dst):
                    st = {}

                    def mm(d2lo):
                        if d2lo == 0:
                            st["ps"] = pp.tile([128, 512], F32, tag="sc",
                                               bufs=4, name="ps_qk")
                        dr_chain(st["ps"], w_sb,
                                 slice(pair * 128, (pair + 1) * 128),
                                 xT_sb, sl, d2lo, 8)

                    def cast():
                        nc.vector.tensor_copy(dst[:, pair, sl], st["ps"][:, :])

                    return [lambda: mm(0), lambda: mm(2), cast]

                return half(wq_sb, qt_all) + half(wk_sb, kt_all)

            # output projection for one (token chunk, D-half)
            def proj_closures(tk, nb):
                st = {}

                def mm():
                    st["ps"] = pp.tile([128, 512], F32, tag="sc", bufs=4,
                                       name="ps_y")
                    if FP8:
                        for c2 in range(2):
                            nc.tensor.matmul(
                                st["ps"][:, :],
                                outT[:, 2 * c2:2 * c2 + 2,
                                     tk * 128:(tk + 1) * 128],
                                wp_sb[:, 2 * c2:2 * c2 + 2,
                                      nb * 512:(nb + 1) * 512],
                                start=(c2 == 0), stop=(c2 == 1),
                                perf_mode=DR,
                            )
                    else:
                        for cc in range(4):
                            nc.tensor.matmul(
                                st["ps"][:, :],
                                outT[:, cc, tk * 128:(tk + 1) * 128],
                                wp_sb[:, cc, nb * 512:(nb + 1) * 512],
                                start=(cc == 0), stop=(cc == 3),
                            )

                def evict():
                    y_sb = work.tile([128, 512], F32, tag="ysb", bufs=3,
                                     name="y_sb")
                    nc.vector.tensor_copy(y_sb[:, :], st["ps"][:, :])
                    nc.sync.dma_start(
                        y[tk * 128:(tk + 1) * 128, nb * 512:(nb + 1) * 512],
                        y_sb[:, :])

                return [mm, evict]

            # prefix: only qg0 pair0+pair1 QK^T emitted directly
            for p in (0, 1):
                for cl in qkt_closures(0, p):
                    cl()

            # V closures live on their own demand-driven queue: AV(kc) drains
            # V up to chunk kc, so V never causes a block-start burst
            vq = []
            vq_mark = {}
            vptr = [0]
            for tk in range(NKC):
                vq.extend(v_tile_closures(tk))
                vq_mark[tk] = len(vq) - 1

            def drain_v(tk):
                tk = min(tk, NKC - 1)
                while vptr[0] <= vq_mark[tk]:
                    vq[vptr[0]]()
                    vptr[0] += 1

            # main filler queue (QK^T projections + output projections)
            marker = {(0, 0): -1, (0, 1): -1}
            for p in (2, 3):
                filler.extend(qkt_closures(0, p))
                marker[(0, p)] = len(filler) - 1
            for qg in (1, 2, 3):
                for p in range(4):
                    filler.extend(qkt_closures(qg, p))
                    marker[(qg, p)] = len(filler) - 1

            # ---- attention: one flat pipeline over (block, k-chunk) ----
            blocks = [(qg, blk) for qg in range(NQG) for blk in (0, 1)]

            def bparams(bi):
                qg, blk = blocks[bi]
                return qg, blk, qg * (TQ // TK), (qg + 1) * (TQ // TK)

            normq = []
            proj_pending = [None]
            psO_of = {}

            def alloc_psO(bi):
                psO_of[bi] = (
                    pp.tile([128, 1024], F32, tag="o", bufs=2, name="psO_A"),
                    pp.tile([128, 1024], F32, tag="o", bufs=2, name="psO_B"),
                )

            def subs_of(bi):
                qg, blk, _, _ = bparams(bi)
                pA, pB = 2 * blk, 2 * blk + 1
                a, b = psO_of[bi]
                return [(pA, 0, a), (pA, 1, a), (pB, 0, b), (pB, 1, b)]

            def qk1(bi, pair, h, kc):
                qg, _, noff, _ = bparams(bi)
                off = max(0, kc - noff) * TK
                ps_s = pp.tile([128, 512], F32, tag="sc", bufs=4, name="ps_s")
                nc.tensor.matmul(
                    ps_s[:, off:512],
                    kt_all[h * 64:(h + 1) * 64, pair, kc * TK:(kc + 1) * TK],
                    qt_all[h * 64:(h + 1) * 64, pair,
                           qg * TQ + off:(qg + 1) * TQ],
                    start=True, stop=True,
                )
                return ps_s

            def exp_mask1(bi, pair, h, kc, ps_s):
                qg, _, noff, _ = bparams(bi)
                off = max(0, kc - noff) * TK
                ex = work.tile([128, 512], BF, tag="ex", bufs=16, name="ex")
                nc.scalar.activation(
                    ex[:, off:512], ps_s[:, off:512],
                    mybir.ActivationFunctionType.Exp,
                    scale=float(EXP_SCALE),
                )
                if kc >= noff:
                    nc.gpsimd.affine_select(
                        ex[:, off:off + TK], ex[:, off:off + TK],
                        pattern=[[1, TK]],
                        compare_op=mybir.AluOpType.is_ge,
                        fill=0.0, base=0, channel_multiplier=-1,
                    )
                return ex

            def av1(bi, pair, h, kc, ex, psO):
                qg, _, noff, kmax = bparams(bi)
                off = max(0, kc - noff) * TK
                nc.tensor.matmul(
                    psO[:, h * 512 + off:(h + 1) * 512],
                    vext[:, kc, pair * 2 + h, :],
                    ex[:, off:512],
                    start=(kc == 0), stop=(kc == kmax - 1),
                    skip_group_check=True,
                )

            def finish_pair(bi, pair, psO, tail=False):
                qg, _, _, _ = bparams(bi)
                st = {}

                def c_den():
                    st["den"] = work.tile([64, 1024], F32, tag="den",
                                          bufs=2, name="den_sb")
                    if tail:
                        nc.scalar.copy(st["den"][:, :], psO[64:128, :])
                    else:
                        nc.vector.tensor_copy(st["den"][:, :], psO[64:128, :])

                def c_recip():
                    st["bcr"] = work.tile([64, 1024], F32, tag="bcr",
                                          bufs=2, name="bcr")
                    nc.vector.reciprocal_approx_fast(st["bcr"][:, :],
                                                     st["den"][:, :])

                def c_mult(h):
                    nc.vector.tensor_mul(
                        outT[h * 64:(h + 1) * 64, pair,
                             qg * TQ:(qg + 1) * TQ],
                        psO[0:64, h * 512:(h + 1) * 512],
                        st["bcr"][0:64, h * 512:(h + 1) * 512],
                    )

                normq.extend([c_den, c_recip,
                              lambda: c_mult(0), lambda: c_mult(1)])

            NB = len(blocks)
            alloc_psO(0)
            cur = [qk1(0, p, h, 0) for (p, h, _) in subs_of(0)]
            cur_next = None
            prev = None  # (bi, kc, exs)
            git = 0      # global iteration counter

            for bi in range(NB):
                qg, blk, noff, kmax = bparams(bi)
                if blk == 0 and qg > 0:
                    proj_pending[0] = list(range((qg - 1) * 4, qg * 4))
                for kc in range(kmax):
                    # V projections the upcoming AVs need, drained on demand
                    drain_v(kc)
                    exs = [exp_mask1(bi, p, h, kc, cur[i])
                           for i, (p, h, _) in enumerate(subs_of(bi))]
                    if prev is not None:
                        pbi, pkc, pexs = prev
                        for i, (p, h, psO) in enumerate(subs_of(pbi)):
                            av1(pbi, p, h, pkc, pexs[i], psO)
                        if pkc == bparams(pbi)[3] - 1:
                            _, pblk, _, _ = bparams(pbi)
                            finish_pair(pbi, 2 * pblk, psO_of[pbi][0])
                            finish_pair(pbi, 2 * pblk + 1, psO_of[pbi][1])
                    for _ in range(4):
                        if normq:
                            normq.pop(0)()
                    if not normq and proj_pending[0] is not None:
                        for tk in proj_pending[0]:
                            for nb in (0, 1):
                                filler.extend(proj_closures(tk, nb))
                        proj_pending[0] = None
                    pump(4 if git < 4 else (3 if qg <= 1 else 2))
                    # two-iteration lookahead: next block's first QKs are
                    # emitted one iteration before this block ends
                    if kc == kmax - 2 and bi + 1 < NB:
                        nqg, nblk, _, _ = bparams(bi + 1)
                        drain_until(max(marker[(nqg, 2 * nblk)],
                                        marker[(nqg, 2 * nblk + 1)]))
                    if kc + 1 < kmax:
                        cur = [qk1(bi, p, h, kc + 1)
                               for (p, h, _) in subs_of(bi)]
                    elif bi + 1 < NB:
                        alloc_psO(bi + 1)
                        cur = [qk1(bi + 1, p, h, 0)
                               for (p, h, _) in subs_of(bi + 1)]
                    else:
                        cur = None
                    prev = (bi, kc, exs)
                    git += 1

            # ---- tail: last AVs + last normalize + last qg's projection ----
            pbi, pkc, pexs = prev
            for i, (p, h, psO) in enumerate(subs_of(pbi)):
                av1(pbi, p, h, pkc, pexs[i], psO)
            finish_pair(pbi, 2, psO_of[pbi][0], tail=True)
            finish_pair(pbi, 3, psO_of[pbi][1])
            while normq:
                normq.pop(0)()
            for tk in range(12, 16):
                for nb in (0, 1):
                    filler.extend(proj_closures(tk, nb))
            drain_until(len(filler) - 1)

    nc.compile()
    return nc


_NC_CACHE = None


def _get_nc():
    global _NC_CACHE
    if _NC_CACHE is None:
        _NC_CACHE = build_nc()
    return _NC_CACHE


def make_in_maps(x, w_qkv, w_proj):
    """Host-side sharding: core c -> (batch c//2, head-group c%2)."""
    np_dt = F8_NP if FP8 else BF_NP
    in_maps = []
    for c in range(N_CORES):
        b, g = divmod(c, 2)
        sl = slice(g * 512, (g + 1) * 512)
        xT = np.ascontiguousarray(x[b].T).astype(np_dt)
        wq = (w_qkv[:, 0 * D:1 * D][:, sl] * WS).astype(np_dt)
        wk = (w_qkv[:, 1 * D:2 * D][:, sl] * WS).astype(np_dt)
        wv = (w_qkv[:, 2 * D:3 * D][:, sl] * WS).astype(np_dt)
        wp = (np.ascontiguousarray(w_proj[sl, :]) * WS).astype(np_dt)
        in_maps.append({"xT": xT, "wq": wq, "wk": wk, "wv": wv, "wp": wp})
    return in_maps


def kernel(x, w_qkv, w_proj, _trace=False, _tmpdir=None):
    x = np.asarray(x, dtype=np.float32)
    w_qkv = np.asarray(w_qkv, dtype=np.float32)
    w_proj = np.asarray(w_proj, dtype=np.float32)
    nc = _get_nc()
    in_maps = make_in_maps(x, w_qkv, w_proj)
    res = run_bass_kernel_spmd(
        nc, in_maps, core_ids=list(range(N_CORES)), trace=_trace, tmpdir=_tmpdir
    )
    out = np.empty((B, T, D), dtype=np.float32)
    inv = np.float32(1.0 / Y_DIV)
    for b in range(B):
        out[b] = (res.results[2 * b]["y"] + res.results[2 * b + 1]["y"]) * inv
    if _trace:
        kernel._last_results = res
    return out


# revision 17
# speedup vs baseline: 1.0172x; 1.0155x over previous
"""Causal multi-head attention (B=4, T=2048, D=1024, H=16) on 8 trn2 cores.

Sharding: core c -> (batch b = c//2, head-group g = c%2) -> 8 heads/core.

Per-core schedule:
  - One FLAT software pipeline over (qg, pair-pair-block, k-chunk): two
    interleaved pair-streams, each with per-head [128,512] score tiles on a
    4-buf PSUM rotation -> 4 exp->QK round-trips overlap and blocks chain
    into each other without pipeline drains.
  - All projection matmuls (QKV, output) run in FP8 E4M3 with DoubleRow
    (2 k-tiles per pass) as "filler" pumped between attention chunks.
    Weights are scaled x8 on host to stay in e4m3 normal range; the softmax
    1/sqrt(dh) and the x64 output compensation fold into the ACT exp scale
    and the host-side gather.
  - V carries 64 ones-columns so the AV matmul materializes the softmax
    denominator replicated across PSUM partitions 64..127; normalization is
    tensor_copy + reciprocal_approx_fast + two fused multiply-evictions,
    dribbled into the following iterations.
  - causal mask applied by gpsimd affine_select directly on ex.
"""

import numpy as np
import ml_dtypes

import concourse.bass as bass  # noqa: F401  (bass types via bacc)
import concourse.bacc as bacc
import concourse.mybir as mybir
import concourse.tile as tile
from concourse.bass_utils import run_bass_kernel_spmd

B, T, D = 4, 2048, 1024
H, DH = 16, 64
N_CORES = 8
HPC = 8      # heads per core
PAIRS = HPC // 2
BF = mybir.dt.bfloat16
F32 = mybir.dt.float32
F8 = mybir.dt.float8e4
BF_NP = ml_dtypes.bfloat16
F8_NP = ml_dtypes.float8_e4m3

FP8 = False
WS = 8.0 if FP8 else 1.0           # host-side weight scale (e4m3 range)
EXP_SCALE = 1.0 / (np.sqrt(DH) * WS * WS)
Y_DIV = WS * WS                    # host-side output divisor
XW = F8 if FP8 else BF
DR = mybir.MatmulPerfMode.DoubleRow if FP8 else None

TQ = 512     # q block (free dim)
TK = 128     # k block (partition dim)
NQG = T // TQ
NKC = T // TK


def build_nc():
    nc = bacc.Bacc(
        "TRN2",
        target_bir_lowering=False,
        debug=False,
        enable_asserts=True,
        num_devices=N_CORES,
    )
    xT = nc.dram_tensor("xT", [D, T], XW, kind="ExternalInput")
    wq = nc.dram_tensor("wq", [D, 512], XW, kind="ExternalInput")
    wk = nc.dram_tensor("wk", [D, 512], XW, kind="ExternalInput")
    wv = nc.dram_tensor("wv", [D, 512], XW, kind="ExternalInput")
    wp = nc.dram_tensor("wp", [512, D], XW, kind="ExternalInput")
    y = nc.dram_tensor("y", [T, D], F32, kind="ExternalOutput")

    with tile.TileContext(nc) as tc:
        with (
            tc.tile_pool(name="pers", bufs=1) as pers,
            tc.tile_pool(name="work", bufs=1) as work,
            tc.tile_pool(name="ps", bufs=1, space="PSUM") as pp,
        ):
            # ---- persistent SBUF ----
            xT_sb = pers.tile([128, 8, T], XW, tag="xT", name="xT_sb")
            wq_sb = pers.tile([128, 8, 512], XW, tag="wq", name="wq_sb")
            wk_sb = pers.tile([128, 8, 512], XW, tag="wk", name="wk_sb")
            wv_sb = pers.tile([128, 8, 512], XW, tag="wv", name="wv_sb")
            wp_sb = pers.tile([128, 4, D], XW, tag="wp", name="wp_sb")
            # V in token-major layout; cols 64..127 are ones so the AV
            # matmul writes the denominator to psO partitions 64..127.
            vext = pers.tile([128, NKC, HPC, 128], BF, tag="vext", name="vext")
            qt_all = pers.tile([128, PAIRS, T], BF, tag="qt", name="qt_all")
            kt_all = pers.tile([128, PAIRS, T], BF, tag="kt", name="kt_all")
            # normalized attention outputs, d-major: [pair-chan, pair, tok]
            outT = pers.tile([128, PAIRS, T], XW, tag="outT", name="outT")

            # ---- loads (ordered so qg0 pair0/1 QK^T deps land first) ----
            for dc in range(8):
                nc.sync.dma_start(wq_sb[:, dc, :], wq[dc * 128:(dc + 1) * 128, :])
                nc.sync.dma_start(wk_sb[:, dc, :], wk[dc * 128:(dc + 1) * 128, :])
                nc.sync.dma_start(xT_sb[:, dc, 0:512], xT[dc * 128:(dc + 1) * 128, 0:512])
            for dc in range(8):
                nc.sync.dma_start(wv_sb[:, dc, :], wv[dc * 128:(dc + 1) * 128, :])
            for dc in range(8):
                nc.sync.dma_start(xT_sb[:, dc, 512:T], xT[dc * 128:(dc + 1) * 128, 512:T])
            for cc in range(4):
                nc.sync.dma_start(wp_sb[:, cc, :], wp[cc * 128:(cc + 1) * 128, :])
            nc.gpsimd.memset(vext[:, :, :, 64:128], 1.0)

            # ---- filler machinery (all PSUM tiles are 1-bank [128,512]) ----
            filler = []
            fptr = [0]

            def pump(n=1):
                while n > 0 and fptr[0] < len(filler):
                    filler[fptr[0]]()
                    fptr[0] += 1
                    n -= 1

            def drain_until(idx):
                while fptr[0] <= idx:
                    if fptr[0] >= len(filler):
                        return
                    filler[fptr[0]]()
                    fptr[0] += 1

            def dr_chain(ps, lhs_t, lhs_cols, rhs_t, rhs_cols, d2lo, ntiles):
                """Two DoubleRow (or four bf16) accumulation matmuls."""
                if FP8:
                    for d2 in range(d2lo, d2lo + 2):
                        nc.tensor.matmul(
                            ps[:, :],
                            lhs_t[:, 2 * d2:2 * d2 + 2, lhs_cols],
                            rhs_t[:, 2 * d2:2 * d2 + 2, rhs_cols],
                            start=(d2 == 0), stop=(d2 == ntiles // 2 - 1),
                            perf_mode=DR,
                        )
                else:
                    for dc in range(2 * d2lo, 2 * d2lo + 4):
                        nc.tensor.matmul(
                            ps[:, :],
                            lhs_t[:, dc, lhs_cols],
                            rhs_t[:, dc, rhs_cols],
                            start=(dc == 0), stop=(dc == ntiles - 1),
                        )

            # V projection of one token chunk tk (all heads)
            def v_tile_closures(tk):
                st = {}

                def mm(d2lo):
                    if d2lo == 0:
                        st["ps"] = pp.tile([128, 512], F32, tag="sc", bufs=4,
                                           name="ps_v")
                    dr_chain(st["ps"], xT_sb, slice(tk * 128, (tk + 1) * 128),
                             wv_sb, slice(0, 512), d2lo, 8)

                def cast():
                    nc.vector.tensor_copy(
                        vext[:, tk, :, 0:64],
                        st["ps"].rearrange("p (h d) -> p h d", d=64),
                    )

                return [lambda: mm(0), lambda: mm(2), cast]

            # Q^T or K^T projection for (qg, pair), d-major, 512 tokens
            def qkt_closures(qg, pair):
                sl = slice(qg * TQ, (qg + 1) * TQ)

                def half(w_sb, dst):
                    st = {}

                    def mm(d2lo):
                        if d2lo == 0:
                            st["ps"] = pp.tile([128, 512], F32, tag="sc",
                                               bufs=4, name="ps_qk")
                        dr_chain(st["ps"], w_sb,
                                 slice(pair * 128, (pair + 1) * 128),
                                 xT_sb, sl, d2lo, 8)

                    def cast():
                        nc.vector.tensor_copy(dst[:, pair, sl], st["ps"][:, :])

                    return [lambda: mm(0), lambda: mm(2), cast]

                return half(wq_sb, qt_all) + half(wk_sb, kt_all)

            # output projection for one (token chunk, D-half)
            def proj_closures(tk, nb):
                st = {}

                def mm():
                    st["ps"] = pp.tile([128, 512], F32, tag="sc", bufs=4,
                                       name="ps_y")
                    if FP8:
                        for c2 in range(2):
                            nc.tensor.matmul(
                                st["ps"][:, :],
                                outT[:, 2 * c2:2 * c2 + 2,
                                     tk * 128:(tk + 1) * 128],
                                wp_sb[:, 2 * c2:2 * c2 + 2,
                                      nb * 512:(nb + 1) * 512],
                                start=(c2 == 0), stop=(c2 == 1),
                                perf_mode=DR,
                            )
                    else:
                        for cc in range(4):
                            nc.tensor.matmul(
                                st["ps"][:, :],
                                outT[:, cc, tk * 128:(tk + 1) * 128],
                                wp_sb[:, cc, nb * 512:(nb + 1) * 512],
                                start=(cc == 0), stop=(cc == 3),
                            )

                def evict():
                    y_sb = work.tile([128, 512], F32, tag="ysb", bufs=3,
                                     name="y_sb")
                    nc.vector.tensor_copy(y_sb[:, :], st["ps"][:, :])
                    nc.sync.dma_start(
                        y[tk * 128:(tk + 1) * 128, nb * 512:(nb + 1) * 512],
                        y_sb[:, :])

                return [mm, evict]

            # prefix: only qg0 pair0+pair1 QK^T emitted directly
            for p in (0, 1):
                for cl in qkt_closures(0, p):
                    cl()

            # V closures live on their own demand-driven queue: AV(kc) drains
            # V up to chunk kc, so V never causes a block-start burst
            vq = []
            vq_mark = {}
            vptr = [0]
            for tk in range(NKC):
                vq.extend(v_tile_closures(tk))
                vq_mark[tk] = len(vq) - 1

            def drain_v(tk):
                tk = min(tk, NKC - 1)
                while vptr[0] <= vq_mark[tk]:
                    vq[vptr[0]]()
                    vptr[0] += 1

            # main filler queue (QK^T projections + output projections)
            marker = {(0, 0): -1, (0, 1): -1}
            for p in (2, 3):
                filler.extend(qkt_closures(0, p))
                marker[(0, p)] = len(filler) - 1
            for qg in (1, 2, 3):
                for p in range(4):
                    filler.extend(qkt_closures(qg, p))
                    marker[(qg, p)] = len(filler) - 1

            # ---- attention: one flat pipeline over (block, k-chunk) ----
            blocks = [(qg, blk) for qg in range(NQG) for blk in (0, 1)]

            def bparams(bi):
                qg, blk = blocks[bi]
                return qg, blk, qg * (TQ // TK), (qg + 1) * (TQ // TK)

            normq = []
            proj_pending = [None]
            psO_of = {}

            def alloc_psO(bi):
                psO_of[bi] = (
                    pp.tile([128, 1024], F32, tag="o", bufs=2, name="psO_A"),
                    pp.tile([128, 1024], F32, tag="o", bufs=2, name="psO_B"),
                )

            def subs_of(bi):
                qg, blk, _, _ = bparams(bi)
                pA, pB = 2 * blk, 2 * blk + 1
                a, b = psO_of[bi]
                return [(pA, 0, a), (pA, 1, a), (pB, 0, b), (pB, 1, b)]

            def qk1(bi, pair, h, kc):
                qg, _, noff, _ = bparams(bi)
                off = max(0, kc - noff) * TK
                ps_s = pp.tile([128, 512], F32, tag="sc", bufs=4, name="ps_s")
                nc.tensor.matmul(
                    ps_s[:, off:512],
                    kt_all[h * 64:(h + 1) * 64, pair, kc * TK:(kc + 1) * TK],
                    qt_all[h * 64:(h + 1) * 64, pair,
                           qg * TQ + off:(qg + 1) * TQ],
                    start=True, stop=True,
                )
                return ps_s

            def exp_mask1(bi, pair, h, kc, ps_s):
                qg, _, noff, _ = bparams(bi)
                off = max(0, kc - noff) * TK
                ex = work.tile([128, 512], BF, tag="ex", bufs=16, name="ex")
                nc.scalar.activation(
                    ex[:, off:512], ps_s[:, off:512],
                    mybir.ActivationFunctionType.Exp,
                    scale=float(EXP_SCALE),
                )
                if kc >= noff:
                    nc.gpsimd.affine_select(
                        ex[:, off:off + TK], ex[:, off:off + TK],
                        pattern=[[1, TK]],
                        compare_op=mybir.AluOpType.is_ge,
                        fill=0.0, base=0, channel_multiplier=-1,
                    )
                return ex

            def av1(bi, pair, h, kc, ex, psO):
                qg, _, noff, kmax = bparams(bi)
                off = max(0, kc - noff) * TK
                nc.tensor.matmul(
                    psO[:, h * 512 + off:(h + 1) * 512],
                    vext[:, kc, pair * 2 + h, :],
                    ex[:, off:512],
                    start=(kc == 0), stop=(kc == kmax - 1),
                    skip_group_check=True,
                )

            def finish_pair(bi, pair, psO, tail=False):
                qg, _, _, _ = bparams(bi)
                st = {}

                def c_den():
                    st["den"] = work.tile([64, 1024], F32, tag="den",
                                          bufs=2, name="den_sb")
                    if tail:
                        nc.scalar.copy(st["den"][:, :], psO[64:128, :])
                    else:
                        nc.vector.tensor_copy(st["den"][:, :], psO[64:128, :])

                def c_recip():
                    st["bcr"] = work.tile([64, 1024], F32, tag="bcr",
                                          bufs=2, name="bcr")
                    nc.vector.reciprocal_approx_fast(st["bcr"][:, :],
                                                     st["den"][:, :])

                def c_mult(h):
                    nc.vector.tensor_mul(
                        outT[h * 64:(h + 1) * 64, pair,
                             qg * TQ:(qg + 1) * TQ],
                        psO[0:64, h * 512:(h + 1) * 512],
                        st["bcr"][0:64, h * 512:(h + 1) * 512],
                    )

                normq.extend([c_den, c_recip,
                              lambda: c_mult(0), lambda: c_mult(1)])

            NB = len(blocks)
            alloc_psO(0)
            cur = [qk1(0, p, h, 0) for (p, h, _) in subs_of(0)]
            cur_next = None
            prev = None  # (bi, kc, exs)
            git = 0      # global iteration counter

            for bi in range(NB):
                qg, blk, noff, kmax = bparams(bi)
                if blk == 0 and qg > 0:
                    proj_pending[0] = list(range((qg - 1) * 4, qg * 4))
                for kc in range(kmax):
                    exs = [exp_mask1(bi, p, h, kc, cur[i])
                           for i, (p, h, _) in enumerate(subs_of(bi))]
                    if prev is not None:
                        pbi, pkc, pexs = prev
                        for i, (p, h, psO) in enumerate(subs_of(pbi)):
                            av1(pbi, p, h, pkc, pexs[i], psO)
                        if pkc == bparams(pbi)[3] - 1:
                            _, pblk, _, _ = bparams(pbi)
                            finish_pair(pbi, 2 * pblk, psO_of[pbi][0])
                            finish_pair(pbi, 2 * pblk + 1, psO_of[pbi][1])
                    for _ in range(4):
                        if normq:
                            normq.pop(0)()
                    if not normq and proj_pending[0] is not None:
                        for tk in proj_pending[0]:
                            for nb in (0, 1):
                                filler.extend(proj_closures(tk, nb))
                        proj_pending[0] = None
                    pump(4 if git < 4 else (3 if qg <= 1 else 2))
                    # two-iteration lookahead: next block's first QKs are
                    # emitted one iteration before this block ends
                    if kc == kmax - 2 and bi + 1 < NB:
                        nqg, nblk, _, _ = bparams(bi + 1)
                        drain_until(max(marker[(nqg, 2 * nblk)],
                                        marker[(nqg, 2 * nblk + 1)]))
                    if kc + 1 < kmax:
                        cur = [qk1(bi, p, h, kc + 1)
                               for (p, h, _) in subs_of(bi)]
                    elif bi + 1 < NB:
                        alloc_psO(bi + 1)
                        cur = [qk1(bi + 1, p, h, 0)
                               for (p, h, _) in subs_of(bi + 1)]
                    else:
                        cur = None
                    # V projections the NEXT iteration's AVs need, emitted
                    # after the latency-critical QKs
                    drain_v(kc + 1)
                    prev = (bi, kc, exs)
                    git += 1

            # ---- tail: last AVs + last normalize + last qg's projection ----
            pbi, pkc, pexs = prev
            for i, (p, h, psO) in enumerate(subs_of(pbi)):
                av1(pbi, p, h, pkc, pexs[i], psO)
            finish_pair(pbi, 2, psO_of[pbi][0], tail=True)
            finish_pair(pbi, 3, psO_of[pbi][1])
            while normq:
                normq.pop(0)()
            for tk in range(12, 16):
                for nb in (0, 1):
                    filler.extend(proj_closures(tk, nb))
            drain_until(len(filler) - 1)

    nc.compile()
    return nc


_NC_CACHE = None


def _get_nc():
    global _NC_CACHE
    if _NC_CACHE is None:
        _NC_CACHE = build_nc()
    return _NC_CACHE


def make_in_maps(x, w_qkv, w_proj):
    """Host-side sharding: core c -> (batch c//2, head-group c%2)."""
    np_dt = F8_NP if FP8 else BF_NP
    in_maps = []
    for c in range(N_CORES):
        b, g = divmod(c, 2)
        sl = slice(g * 512, (g + 1) * 512)
        xT = np.ascontiguousarray(x[b].T).astype(np_dt)
        wq = (w_qkv[:, 0 * D:1 * D][:, sl] * WS).astype(np_dt)
        wk = (w_qkv[:, 1 * D:2 * D][:, sl] * WS).astype(np_dt)
        wv = (w_qkv[:, 2 * D:3 * D][:, sl] * WS).astype(np_dt)
        wp = (np.ascontiguousarray(w_proj[sl, :]) * WS).astype(np_dt)
        in_maps.append({"xT": xT, "wq": wq, "wk": wk, "wv": wv, "wp": wp})
    return in_maps


def kernel(x, w_qkv, w_proj, _trace=False, _tmpdir=None):
    x = np.asarray(x, dtype=np.float32)
    w_qkv = np.asarray(w_qkv, dtype=np.float32)
    w_proj = np.asarray(w_proj, dtype=np.float32)
    nc = _get_nc()
    in_maps = make_in_maps(x, w_qkv, w_proj)
    res = run_bass_kernel_spmd(
        nc, in_maps, core_ids=list(range(N_CORES)), trace=_trace, tmpdir=_tmpdir
    )
    out = np.empty((B, T, D), dtype=np.float32)
    inv = np.float32(1.0 / Y_DIV)
    for b in range(B):
        out[b] = (res.results[2 * b]["y"] + res.results[2 * b + 1]["y"]) * inv
    if _trace:
        kernel._last_results = res
    return out
